# revision 9
# baseline (speedup 1.0000x reference)
"""Trainium2 Bass kernel for nn_Decay (gated decay-memory block).

  gate  = sigmoid(x @ Wg + bg)
  store = (x @ Wv) * gate * scale          scale = sqrt(1 - decay)
  mem   = decay-scan(store)                y_t = store_t + decay * y_{t-1}
  que   = sigmoid(x @ Wq + bq)
  out   = (mem * que * scale) @ Wo

Sharding (8 cores): core c handles batch b = c//2, token half h = c%2
(2048 output tokens each).  The decay scan needs history: each core
computes a 256-token halo before its token range (zero-padded for h=0,
so all cores run the identical program).  decay^256 ~ 2e-6.  No
collectives.

Precision plan (tolerance rel 2e-2; predicted 1.39e-2 on real inputs):
 - V path and O path in bf16 (error contribution ~3e-3)
 - gate/que GEMMs: K rows 0..KF-1 in fp8 e4m3 with DoubleRow perf mode
   (1.44x PE rate), remainder rows in bf16.  Both fp8 operands are
   pre-scaled by 8 host-side (product 64) and the bf16 remainder weights
   by 64, so one PSUM accumulates 64*z; the sigmoid applies scale=1/64.
 - scan state/input fp32; que/l0/weights/x bf16; PSUM fp32; out fp32.

Layout: on-chip [feature (partitions), token (free)].  Free dim 512
(halo block 256) so f32-era LDWEIGHTS leak is amortized; bf16 gets FWL.

Schedule: 4 m-quarter phases x 5 token blocks; que-projection (pq) for
block i runs during block i+1 (and the last block's pq drains into the
next phase / the C transition), so wq loads and phase-boundary weight
loads always have a full block of PE work as cover.  Phase C (output
projection) keeps all four Wo e-quarters resident by reusing SBUF tag
space freed by the A-phase weights.
"""

import sys

sys.path.insert(0, "/opt/trn_rl_repo")

import ml_dtypes
import numpy as np

import concourse.bass as bass
import concourse.tile as tile
from concourse import bacc, mybir
from concourse.bass_utils import run_bass_kernel_spmd

# Problem constants (hardcoded per harness contract)
B, S, E, M = 4, 4096, 2048, 2048
DECAY = 0.95
SCALE = float(np.sqrt(1.0 - DECAY))

N_CORES = 8
HALO = 256
OUT_T = S // 2        # 2048 output tokens per core
T = OUT_T + HALO      # 2304 computed tokens per core
P = 128
KF = 1536             # fp8 K-prefix for gate/que projections
KR = E - KF
EC8 = KF // 256       # DoubleRow k-pair count (4)
ECR = KR // 128       # bf16 remainder k-chunks (8)
EC = E // P           # 16
MT = M // P           # 16
MQ = 4                # m-quarter phases
MT_Q = MT // MQ       # 4 m-tiles per quarter
MQW = MT_Q * P        # 512
BLK = [(0, 256), (256, 512), (768, 512), (1280, 512), (1792, 512)]
NB = len(BLK)
XS = 8.0              # fp8 per-operand scale (product 64)
WS = XS * XS
F32 = mybir.dt.float32
BF16 = mybir.dt.bfloat16
FP8 = mybir.dt.float8e4
DR = mybir.MatmulPerfMode.DoubleRow
SIG = mybir.ActivationFunctionType.Sigmoid


def build_module(has_bias):
    nc = bacc.Bacc()

    xT_d = nc.dram_tensor("xT16", [E, T], BF16, kind="ExternalInput")
    x8_d = nc.dram_tensor("xT8", [KF, T], FP8, kind="ExternalInput")
    wv_d = nc.dram_tensor("Wv16", [E, M], BF16, kind="ExternalInput")
    wg8_d = nc.dram_tensor("Wg8", [KF, M], FP8, kind="ExternalInput")
    wg16_d = nc.dram_tensor("Wg16", [KR, M], BF16, kind="ExternalInput")
    wq8_d = nc.dram_tensor("Wq8", [KF, M], FP8, kind="ExternalInput")
    wq16_d = nc.dram_tensor("Wq16", [KR, M], BF16, kind="ExternalInput")
    wo_d = nc.dram_tensor("Wo16", [M, E], BF16, kind="ExternalInput")
    if has_bias:
        bg_d = nc.dram_tensor("bg", [M], F32, kind="ExternalInput")
        bq_d = nc.dram_tensor("bq", [M], F32, kind="ExternalInput")
    outT_d = nc.dram_tensor("outT", [E, OUT_T], F32, kind="ExternalOutput")
    l0_d = nc.dram_tensor("l0buf", [M, OUT_T], BF16)  # internal spill

    xT_r = xT_d.rearrange("(c p) t -> p c t", p=P)
    x8_r = x8_d.rearrange("(c j p) t -> p c j t", p=P, j=2)
    l0_r = l0_d.rearrange("(c p) t -> p c t", p=P)
    outT_r = outT_d.rearrange("(c p) t -> p c t", p=P)

    with tile.TileContext(nc) as tc:
        with (
            tc.tile_pool(name="w", bufs=2) as wp,
            tc.tile_pool(name="a", bufs=2) as sp,
            tc.tile_pool(name="ps", bufs=2, space="PSUM") as ps,
        ):
            consts = sp.tile([P, 512 + 2 * MT], F32, tag="consts", bufs=1)
            nc.vector.memset(consts[:, 0:512], DECAY)
            if has_bias:
                nc.sync.dma_start(
                    out=consts[:, 512 : 512 + MT],
                    in_=bg_d.rearrange("(c p) -> p c", p=P),
                )
                nc.sync.dma_start(
                    out=consts[:, 512 + MT : 512 + 2 * MT],
                    in_=bq_d.rearrange("(c p) -> p c", p=P),
                )
            decay_t = consts[:, 0:512]

            def bias_ap(kind, mtg):
                if not has_bias:
                    return 0.0
                off = 512 + (0 if kind == "g" else MT) + mtg
                return consts[:, off : off + 1]

            W16 = [P, EC, MQW]  # 16KB/partition: wv / wo / (padded) wg16, wq16

            def load_wv(q):
                t = wp.tile(W16, BF16, tag="wv", name=f"wv{q}")
                nc.scalar.dma_start(
                    out=t,
                    in_=wv_d[:, q * MQW : (q + 1) * MQW].rearrange(
                        "(c p) m -> p c m", p=P
                    ),
                )
                return t

            def load_w8(d, q, tag, nm, eng=None):
                t = wp.tile([P, EC8, 2, MQW], FP8, tag=tag, name=nm)
                (eng or nc.scalar).dma_start(
                    out=t,
                    in_=d[:, q * MQW : (q + 1) * MQW].rearrange(
                        "(c j p) m -> p c j m", p=P, j=2
                    ),
                )
                return t

            def load_w16(d, q, tag, nm, eng=None):
                t = wp.tile([P, ECR, MQW], BF16, tag=tag, name=nm)
                (eng or nc.scalar).dma_start(
                    out=t,
                    in_=d[:, q * MQW : (q + 1) * MQW].rearrange(
                        "(c p) m -> p c m", p=P
                    ),
                )
                return t

            def load_wo(eq, tag):
                t = wp.tile(W16, BF16, tag=tag, name=f"wo{eq}")
                nc.scalar.dma_start(
                    out=t,
                    in_=wo_d[:, eq * MQW : (eq + 1) * MQW].rearrange(
                        "(c p) e -> p c e", p=P
                    ),
                )
                return t

            def load_x(s):
                # x8 first: the first block's DR matmuls need only x8 + wg8
                q, i = divmod(s, NB)
                t0, tsz = BLK[i]
                x8t = sp.tile(
                    [P, EC8, 2, 512], FP8, tag="x8", bufs=3, name=f"x8_{q}_{i}"
                )
                nc.sync.dma_start(
                    out=x8t[:, :, :, :tsz], in_=x8_r[:, :, :, t0 : t0 + tsz]
                )
                xt = sp.tile([P, EC, 512], BF16, tag="xt", bufs=3, name=f"xt{q}_{i}")
                nc.sync.dma_start(out=xt[:, :, :tsz], in_=xT_r[:, :, t0 : t0 + tsz])
                return xt, x8t

            def emit_pq(inf):
                # deferred que-projection + load for a previous block
                tszp = inf["tsz"]
                for mt in range(MT_Q):
                    msl = slice(mt * P, (mt + 1) * P)
                    mtg = inf["q"] * MT_Q + mt
                    pqp = ps.tile(
                        [P, 512], F32, tag="pq", bufs=2,
                        name=f"pq{inf['q']}_{inf['i']}_{mt}",
                    )[:, :tszp]
                    for c2 in range(EC8):
                        nc.tensor.matmul(
                            pqp, lhsT=inf["q8"][:, c2, :, msl],
                            rhs=inf["x8"][:, c2, :, :tszp],
                            start=(c2 == 0), stop=False, perf_mode=DR,
                        )
                    for ec in range(ECR):
                        nc.tensor.matmul(
                            pqp, lhsT=inf["q16"][:, ec, msl],
                            rhs=inf["xt"][:, 2 * EC8 + ec, :tszp],
                            start=False, stop=(ec == ECR - 1),
                        )
                    que = sp.tile(
                        [P, 512], BF16, tag="que", bufs=3,
                        name=f"que{inf['q']}_{inf['i']}_{mt}",
                    )[:, :tszp]
                    nc.scalar.activation(
                        que, pqp, SIG, bias=bias_ap("q", mtg), scale=1.0 / WS
                    )
                    l0 = sp.tile(
                        [P, 512], BF16, tag="l0", bufs=3,
                        name=f"l0_{inf['q']}_{inf['i']}_{mt}",
                    )[:, :tszp]
                    nc.vector.tensor_mul(l0, inf["mem"][:, mt, :tszp], que)
                    nc.gpsimd.dma_start(
                        out=l0_r[:, mtg : mtg + 1, inf["osl"]], in_=l0.unsqueeze(1)
                    )

            # ---- Phase A: 4 m-quarters x 5 token blocks ----
            steps = [(q, i) for q in range(MQ) for i in range(NB)]
            xts = {0: load_x(0)}
            # startup: spread the q0 weight loads over distinct DMA queues so
            # the first blocks' pv/pg aren't serialized behind one queue
            cur = {
                "wv": load_wv(0),
                "g8": load_w8(wg8_d, 0, "wg8", "wg8_0", nc.gpsimd),
                "g16": load_w16(wg16_d, 0, "wg16", "wg16_0", nc.gpsimd),
                "q8": load_w8(wq8_d, 0, "wq8", "wq8_0", nc.gpsimd),
                "q16": load_w16(wq16_d, 0, "wq16", "wq16_0", nc.gpsimd),
            }
            nxt = {}
            wo_t = {}
            prev = None      # deferred-pq info from previous block
            mem_prev = None  # previous block's mem (scan chain)

            for s, (q, i) in enumerate(steps):
                t0, tsz = BLK[i]
                if i == 0 and q > 0:
                    cur = nxt
                    nxt = {}
                xt, x8t = xts.pop(s)
                if s + 1 < len(steps):
                    xts[s + 1] = load_x(s + 1)

                # phase-boundary weight prefetches (a quarter ahead / wo)
                if q < MQ - 1:
                    if i == 2:
                        nxt["wv"] = load_wv(q + 1)
                    elif i == 3:
                        nxt["g8"] = load_w8(wg8_d, q + 1, "wg8", f"wg8_{q+1}")
                        nxt["g16"] = load_w16(wg16_d, q + 1, "wg16", f"wg16_{q+1}")
                    elif i == 4:
                        nxt["q8"] = load_w8(wq8_d, q + 1, "wq8", f"wq8_{q+1}")
                        nxt["q16"] = load_w16(wq16_d, q + 1, "wq16", f"wq16_{q+1}")
                else:
                    if i == 1:
                        wo_t[2] = load_wo(2, "wo")
                    elif i == 2:
                        wo_t[0] = load_wo(0, "wv")
                    elif i == 3:
                        wo_t[3] = load_wo(3, "wo")

                # deferred pq for the previous block (keeps PE busy while
                # this block's x/weights stream in)
                if prev is not None:
                    emit_pq(prev)

                # pg first (DR matmuls need only x8 + wg8 — shortest startup
                # dependency), then pv; gates are ready by the time the
                # store-muls consume the pv psums, so pv's ring never cycles
                # into an unmet DVE dependency.
                mem_t = sp.tile(
                    [P, MT_Q, 512], F32, tag="mem", bufs=2, name=f"mem{q}_{i}"
                )
                gates = []
                for mt in range(MT_Q):
                    msl = slice(mt * P, (mt + 1) * P)
                    mtg = q * MT_Q + mt
                    pgp = ps.tile(
                        [P, 512], F32, tag="pg", bufs=2, name=f"pg{q}_{i}_{mt}"
                    )[:, :tsz]
                    for c2 in range(EC8):
                        nc.tensor.matmul(
                            pgp, lhsT=cur["g8"][:, c2, :, msl],
                            rhs=x8t[:, c2, :, :tsz],
                            start=(c2 == 0), stop=False, perf_mode=DR,
                        )
                    for ec in range(ECR):
                        nc.tensor.matmul(
                            pgp, lhsT=cur["g16"][:, ec, msl],
                            rhs=xt[:, 2 * EC8 + ec, :tsz],
                            start=False, stop=(ec == ECR - 1),
                        )
                    gate = sp.tile(
                        [P, 512], BF16, tag="gate", bufs=2, name=f"gate{q}_{i}_{mt}"
                    )[:, :tsz]
                    nc.scalar.activation(
                        gate, pgp, SIG, bias=bias_ap("g", mtg), scale=1.0 / WS
                    )
                    gates.append(gate)
                for mt in range(MT_Q):
                    msl = slice(mt * P, (mt + 1) * P)
                    pvp = ps.tile(
                        [P, 512], F32, tag="pv", bufs=3, name=f"pv{q}_{i}_{mt}"
                    )[:, :tsz]
                    for ec in range(EC):
                        nc.tensor.matmul(
                            pvp, lhsT=cur["wv"][:, ec, msl], rhs=xt[:, ec, :tsz],
                            start=(ec == 0), stop=(ec == EC - 1),
                        )
                    store = sp.tile(
                        [P, 512], F32, tag="store", bufs=2, name=f"st{q}_{i}_{mt}"
                    )[:, :tsz]
                    nc.vector.tensor_mul(store, pvp, gates[mt])
                    init = (
                        0.0
                        if i == 0
                        else mem_prev[:, mt, BLK[i - 1][1] - 1 : BLK[i - 1][1]]
                    )
                    nc.vector.tensor_tensor_scan(
                        mem_t[:, mt, :tsz], decay_t[:, :tsz], store,
                        initial=init,
                        op0=mybir.AluOpType.mult, op1=mybir.AluOpType.add,
                    )

                prev = (
                    None
                    if i == 0
                    else dict(
                        q=q, i=i, tsz=tsz, mem=mem_t, xt=xt, x8=x8t,
                        q8=cur["q8"], q16=cur["q16"],
                        osl=slice(t0 - HALO, t0 - HALO + tsz),
                    )
                )
                mem_prev = mem_t

            emit_pq(prev)  # drain: pq for (3, B4) covers the C transition
            wo_t[1] = load_wo(1, "wv")

            # ---- Phase C: output projection, all Wo quarters resident ----
            lt = sp.tile([P, MT, 512], BF16, tag="xt", bufs=3, name="lt0")
            nc.sync.dma_start(out=lt, in_=l0_r[:, :, 0:512])
            for tb in range(OUT_T // 512):
                tsl = slice(tb * 512, (tb + 1) * 512)
                lt_next = None
                if tb + 1 < OUT_T // 512:
                    lt_next = sp.tile(
                        [P, MT, 512], BF16, tag="xt", bufs=3, name=f"lt{tb+1}"
                    )
                    nc.sync.dma_start(
                        out=lt_next, in_=l0_r[:, :, (tb + 1) * 512 : (tb + 2) * 512]
                    )
                for eq in range(4):
                    ot = sp.tile(
                        [P, MT_Q, 512], F32, tag="mem", bufs=2, name=f"ot{eq}_{tb}"
                    )
                    for et in range(MT_Q):
                        pop = ps.tile(
                            [P, 512], F32, tag="pv", bufs=3, name=f"po{eq}_{tb}_{et}"
                        )
                        for mc in range(MT):
                            nc.tensor.matmul(
                                pop,
                                lhsT=wo_t[eq][:, mc, et * P : (et + 1) * P],
                                rhs=lt[:, mc, :],
                                start=(mc == 0), stop=(mc == MT - 1),
                            )
                        nc.vector.tensor_copy(ot[:, et, :], pop)
                    nc.gpsimd.dma_start(
                        out=outT_r[:, eq * MT_Q : (eq + 1) * MT_Q, tsl], in_=ot
                    )
                lt = lt_next
    nc.compile()
    return nc


_cached = {}


def _get_module(has_bias):
    if has_bias not in _cached:
        _cached[has_bias] = build_module(has_bias)
    return _cached[has_bias]


def _q8(a):
    return np.clip(a * np.float32(XS), -240, 240).astype(ml_dtypes.float8_e4m3)


def _prep_inputs(x, Wv, Wg, bg, Wq, bq, Wo, has_bias):
    """Shard + quantize host-side. Returns per-core input dicts."""
    bf = ml_dtypes.bfloat16
    x = np.asarray(x, dtype=np.float32)
    Wv16 = (np.asarray(Wv, np.float32) * np.float32(SCALE)).astype(bf)
    Wo16 = (np.asarray(Wo, np.float32) * np.float32(SCALE)).astype(bf)
    Wg = np.asarray(Wg, np.float32)
    Wq = np.asarray(Wq, np.float32)
    Wg8, Wq8 = _q8(Wg[:KF]), _q8(Wq[:KF])
    Wg16 = (Wg[KF:] * np.float32(WS)).astype(bf)
    Wq16 = (Wq[KF:] * np.float32(WS)).astype(bf)
    in_maps = []
    for c in range(N_CORES):
        b, h = c // 2, c % 2
        xTc = np.zeros((E, T), dtype=np.float32)
        start = h * OUT_T - HALO
        src = np.ascontiguousarray(x[b, max(start, 0) : h * OUT_T + OUT_T].T)
        xTc[:, T - src.shape[1] :] = src
        m = {
            "xT16": xTc.astype(bf), "xT8": _q8(xTc[:KF]),
            "Wv16": Wv16, "Wg8": Wg8, "Wg16": Wg16,
            "Wq8": Wq8, "Wq16": Wq16, "Wo16": Wo16,
        }
        if has_bias:
            m["bg"] = np.ascontiguousarray(bg, dtype=np.float32)
            m["bq"] = np.ascontiguousarray(bq, dtype=np.float32)
        in_maps.append(m)
    return in_maps


def run(x, Wv, Wg, bg, Wq, bq, Wo, trace=False):
    bg = np.asarray(bg, dtype=np.float32)
    bq = np.asarray(bq, dtype=np.float32)
    has_bias = bool(np.any(bg)) or bool(np.any(bq))
    nc = _get_module(has_bias)
    in_maps = _prep_inputs(x, Wv, Wg, bg, Wq, bq, Wo, has_bias)
    res = run_bass_kernel_spmd(
        nc, in_maps, core_ids=list(range(N_CORES)), trace=trace
    )
    out = np.empty((B, S, E), dtype=np.float32)
    for c in range(N_CORES):
        b, h = c // 2, c % 2
        out[b, h * OUT_T : (h + 1) * OUT_T] = res.results[c]["outT"].T
    return out, res


def kernel(**inputs):
    out, _ = run(**inputs)
    return out


# revision 10
# speedup vs baseline: 1.0071x; 1.0071x over previous
"""Trainium2 Bass kernel for nn_Decay (gated decay-memory block).

  gate  = sigmoid(x @ Wg + bg)
  store = (x @ Wv) * gate * scale          scale = sqrt(1 - decay)
  mem   = decay-scan(store)                y_t = store_t + decay * y_{t-1}
  que   = sigmoid(x @ Wq + bq)
  out   = (mem * que * scale) @ Wo

Sharding (8 cores): core c handles batch b = c//2, token half h = c%2
(2048 output tokens each).  The decay scan needs history: each core
computes a 256-token halo before its token range (zero-padded for h=0,
so all cores run the identical program).  decay^256 ~ 2e-6.  No
collectives.

Precision plan (tolerance rel 2e-2; predicted 1.39e-2 on real inputs):
 - V path and O path in bf16 (error contribution ~3e-3)
 - gate/que GEMMs: K rows 0..KF-1 in fp8 e4m3 with DoubleRow perf mode
   (1.44x PE rate), remainder rows in bf16.  Both fp8 operands are
   pre-scaled by 8 host-side (product 64) and the bf16 remainder weights
   by 64, so one PSUM accumulates 64*z; the sigmoid applies scale=1/64.
 - scan state/input fp32; que/l0/weights/x bf16; PSUM fp32; out fp32.

Layout: on-chip [feature (partitions), token (free)].  Free dim 512
(halo block 256) so f32-era LDWEIGHTS leak is amortized; bf16 gets FWL.

Schedule: 4 m-quarter phases x 5 token blocks; que-projection (pq) for
block i runs during block i+1 (and the last block's pq drains into the
next phase / the C transition), so wq loads and phase-boundary weight
loads always have a full block of PE work as cover.  Phase C (output
projection) keeps all four Wo e-quarters resident by reusing SBUF tag
space freed by the A-phase weights.
"""

import sys

sys.path.insert(0, "/opt/trn_rl_repo")

import ml_dtypes
import numpy as np

import concourse.bass as bass
import concourse.tile as tile
from concourse import bacc, mybir
from concourse.bass_utils import run_bass_kernel_spmd

# Problem constants (hardcoded per harness contract)
B, S, E, M = 4, 4096, 2048, 2048
DECAY = 0.95
SCALE = float(np.sqrt(1.0 - DECAY))

N_CORES = 8
HALO = 256
OUT_T = S // 2        # 2048 output tokens per core
T = OUT_T + HALO      # 2304 computed tokens per core
P = 128
KF = 1536             # fp8 K-prefix for gate/que projections
KR = E - KF
EC8 = KF // 256       # DoubleRow k-pair count (4)
ECR = KR // 128       # bf16 remainder k-chunks (8)
EC = E // P           # 16
MT = M // P           # 16
MQ = 4                # m-quarter phases
MT_Q = MT // MQ       # 4 m-tiles per quarter
MQW = MT_Q * P        # 512
BLK = [(0, 256), (256, 512), (768, 512), (1280, 512), (1792, 512)]
NB = len(BLK)
XS = 8.0              # fp8 per-operand scale (product 64)
WS = XS * XS
F32 = mybir.dt.float32
BF16 = mybir.dt.bfloat16
FP8 = mybir.dt.float8e4
DR = mybir.MatmulPerfMode.DoubleRow
SIG = mybir.ActivationFunctionType.Sigmoid


def build_module(has_bias):
    nc = bacc.Bacc()

    xT_d = nc.dram_tensor("xT16", [E, T], BF16, kind="ExternalInput")
    x8_d = nc.dram_tensor("xT8", [KF, T], FP8, kind="ExternalInput")
    wv_d = nc.dram_tensor("Wv16", [E, M], BF16, kind="ExternalInput")
    wg8_d = nc.dram_tensor("Wg8", [KF, M], FP8, kind="ExternalInput")
    wg16_d = nc.dram_tensor("Wg16", [KR, M], BF16, kind="ExternalInput")
    wq8_d = nc.dram_tensor("Wq8", [KF, M], FP8, kind="ExternalInput")
    wq16_d = nc.dram_tensor("Wq16", [KR, M], BF16, kind="ExternalInput")
    wo_d = nc.dram_tensor("Wo16", [M, E], BF16, kind="ExternalInput")
    if has_bias:
        bg_d = nc.dram_tensor("bg", [M], F32, kind="ExternalInput")
        bq_d = nc.dram_tensor("bq", [M], F32, kind="ExternalInput")
    outT_d = nc.dram_tensor("outT", [E, OUT_T], F32, kind="ExternalOutput")
    l0_d = nc.dram_tensor("l0buf", [M, OUT_T], BF16)  # internal spill

    xT_r = xT_d.rearrange("(c p) t -> p c t", p=P)
    x8_r = x8_d.rearrange("(c j p) t -> p c j t", p=P, j=2)
    l0_r = l0_d.rearrange("(c p) t -> p c t", p=P)
    outT_r = outT_d.rearrange("(c p) t -> p c t", p=P)

    with tile.TileContext(nc) as tc:
        with (
            tc.tile_pool(name="w", bufs=2) as wp,
            tc.tile_pool(name="a", bufs=2) as sp,
            tc.tile_pool(name="ps", bufs=2, space="PSUM") as ps,
        ):
            consts = sp.tile([P, 512 + 2 * MT], F32, tag="consts", bufs=1)
            nc.vector.memset(consts[:, 0:512], DECAY)
            if has_bias:
                nc.sync.dma_start(
                    out=consts[:, 512 : 512 + MT],
                    in_=bg_d.rearrange("(c p) -> p c", p=P),
                )
                nc.sync.dma_start(
                    out=consts[:, 512 + MT : 512 + 2 * MT],
                    in_=bq_d.rearrange("(c p) -> p c", p=P),
                )
            decay_t = consts[:, 0:512]

            def bias_ap(kind, mtg):
                if not has_bias:
                    return 0.0
                off = 512 + (0 if kind == "g" else MT) + mtg
                return consts[:, off : off + 1]

            W16 = [P, EC, MQW]  # 16KB/partition: wv / wo / (padded) wg16, wq16

            def load_wv(q):
                t = wp.tile(W16, BF16, tag="wv", name=f"wv{q}")
                nc.scalar.dma_start(
                    out=t,
                    in_=wv_d[:, q * MQW : (q + 1) * MQW].rearrange(
                        "(c p) m -> p c m", p=P
                    ),
                )
                return t

            def load_w8(d, q, tag, nm, eng=None):
                t = wp.tile([P, EC8, 2, MQW], FP8, tag=tag, name=nm)
                (eng or nc.scalar).dma_start(
                    out=t,
                    in_=d[:, q * MQW : (q + 1) * MQW].rearrange(
                        "(c j p) m -> p c j m", p=P, j=2
                    ),
                )
                return t

            def load_w16(d, q, tag, nm, eng=None):
                t = wp.tile([P, ECR, MQW], BF16, tag=tag, name=nm)
                (eng or nc.scalar).dma_start(
                    out=t,
                    in_=d[:, q * MQW : (q + 1) * MQW].rearrange(
                        "(c p) m -> p c m", p=P
                    ),
                )
                return t

            def load_wo(eq, tag):
                t = wp.tile(W16, BF16, tag=tag, name=f"wo{eq}")
                nc.scalar.dma_start(
                    out=t,
                    in_=wo_d[:, eq * MQW : (eq + 1) * MQW].rearrange(
                        "(c p) e -> p c e", p=P
                    ),
                )
                return t

            def load_x(s):
                # x8 first: the first block's DR matmuls need only x8 + wg8
                q, i = divmod(s, NB)
                t0, tsz = BLK[i]
                x8t = sp.tile(
                    [P, EC8, 2, 512], FP8, tag="x8", bufs=3, name=f"x8_{q}_{i}"
                )
                nc.sync.dma_start(
                    out=x8t[:, :, :, :tsz], in_=x8_r[:, :, :, t0 : t0 + tsz]
                )
                xt = sp.tile([P, EC, 512], BF16, tag="xt", bufs=3, name=f"xt{q}_{i}")
                nc.sync.dma_start(out=xt[:, :, :tsz], in_=xT_r[:, :, t0 : t0 + tsz])
                return xt, x8t

            def emit_pq(inf):
                # deferred que-projection + load for a previous block
                tszp = inf["tsz"]
                for mt in range(MT_Q):
                    msl = slice(mt * P, (mt + 1) * P)
                    mtg = inf["q"] * MT_Q + mt
                    pqp = ps.tile(
                        [P, 512], F32, tag="pq", bufs=2,
                        name=f"pq{inf['q']}_{inf['i']}_{mt}",
                    )[:, :tszp]
                    for c2 in range(EC8):
                        nc.tensor.matmul(
                            pqp, lhsT=inf["q8"][:, c2, :, msl],
                            rhs=inf["x8"][:, c2, :, :tszp],
                            start=(c2 == 0), stop=False, perf_mode=DR,
                        )
                    for ec in range(ECR):
                        nc.tensor.matmul(
                            pqp, lhsT=inf["q16"][:, ec, msl],
                            rhs=inf["xt"][:, 2 * EC8 + ec, :tszp],
                            start=False, stop=(ec == ECR - 1),
                        )
                    que = sp.tile(
                        [P, 512], BF16, tag="que", bufs=3,
                        name=f"que{inf['q']}_{inf['i']}_{mt}",
                    )[:, :tszp]
                    nc.scalar.activation(
                        que, pqp, SIG, bias=bias_ap("q", mtg), scale=1.0 / WS
                    )
                    l0 = sp.tile(
                        [P, 512], BF16, tag="l0", bufs=3,
                        name=f"l0_{inf['q']}_{inf['i']}_{mt}",
                    )[:, :tszp]
                    nc.vector.tensor_mul(l0, inf["mem"][:, mt, :tszp], que)
                    nc.gpsimd.dma_start(
                        out=l0_r[:, mtg : mtg + 1, inf["osl"]], in_=l0.unsqueeze(1)
                    )

            # ---- Phase A: 4 m-quarters x 5 token blocks ----
            steps = [(q, i) for q in range(MQ) for i in range(NB)]
            xts = {0: load_x(0)}
            # startup: spread the q0 weight loads over distinct DMA queues so
            # the first blocks' pv/pg aren't serialized behind one queue
            cur = {
                "g8": load_w8(wg8_d, 0, "wg8", "wg8_0"),
                "g16": load_w16(wg16_d, 0, "wg16", "wg16_0"),
                "wv": load_wv(0),
                "q8": load_w8(wq8_d, 0, "wq8", "wq8_0", nc.gpsimd),
                "q16": load_w16(wq16_d, 0, "wq16", "wq16_0", nc.gpsimd),
            }
            nxt = {}
            wo_t = {}
            prev = None      # deferred-pq info from previous block
            mem_prev = None  # previous block's mem (scan chain)

            for s, (q, i) in enumerate(steps):
                t0, tsz = BLK[i]
                if i == 0 and q > 0:
                    cur = nxt
                    nxt = {}
                xt, x8t = xts.pop(s)
                if s + 1 < len(steps):
                    xts[s + 1] = load_x(s + 1)

                # phase-boundary weight prefetches (a quarter ahead / wo)
                if q < MQ - 1:
                    if i == 2:
                        nxt["wv"] = load_wv(q + 1)
                    elif i == 3:
                        nxt["g8"] = load_w8(wg8_d, q + 1, "wg8", f"wg8_{q+1}")
                        nxt["g16"] = load_w16(wg16_d, q + 1, "wg16", f"wg16_{q+1}")
                    elif i == 4:
                        nxt["q8"] = load_w8(wq8_d, q + 1, "wq8", f"wq8_{q+1}")
                        nxt["q16"] = load_w16(wq16_d, q + 1, "wq16", f"wq16_{q+1}")
                else:
                    if i == 1:
                        wo_t[2] = load_wo(2, "wo")
                    elif i == 2:
                        wo_t[0] = load_wo(0, "wv")
                    elif i == 3:
                        wo_t[3] = load_wo(3, "wo")

                # deferred pq for the previous block (keeps PE busy while
                # this block's x/weights stream in)
                if prev is not None:
                    emit_pq(prev)

                # pg first (DR matmuls need only x8 + wg8 — shortest startup
                # dependency), then pv; gates are ready by the time the
                # store-muls consume the pv psums, so pv's ring never cycles
                # into an unmet DVE dependency.
                mem_t = sp.tile(
                    [P, MT_Q, 512], F32, tag="mem", bufs=2, name=f"mem{q}_{i}"
                )
                gates = []
                for mt in range(MT_Q):
                    msl = slice(mt * P, (mt + 1) * P)
                    mtg = q * MT_Q + mt
                    pgp = ps.tile(
                        [P, 512], F32, tag="pg", bufs=2, name=f"pg{q}_{i}_{mt}"
                    )[:, :tsz]
                    for c2 in range(EC8):
                        nc.tensor.matmul(
                            pgp, lhsT=cur["g8"][:, c2, :, msl],
                            rhs=x8t[:, c2, :, :tsz],
                            start=(c2 == 0), stop=False, perf_mode=DR,
                        )
                    for ec in range(ECR):
                        nc.tensor.matmul(
                            pgp, lhsT=cur["g16"][:, ec, msl],
                            rhs=xt[:, 2 * EC8 + ec, :tsz],
                            start=False, stop=(ec == ECR - 1),
                        )
                    gate = sp.tile(
                        [P, 512], BF16, tag="gate", bufs=2, name=f"gate{q}_{i}_{mt}"
                    )[:, :tsz]
                    nc.scalar.activation(
                        gate, pgp, SIG, bias=bias_ap("g", mtg), scale=1.0 / WS
                    )
                    gates.append(gate)
                for mt in range(MT_Q):
                    msl = slice(mt * P, (mt + 1) * P)
                    pvp = ps.tile(
                        [P, 512], F32, tag="pv", bufs=3, name=f"pv{q}_{i}_{mt}"
                    )[:, :tsz]
                    for ec in range(EC):
                        nc.tensor.matmul(
                            pvp, lhsT=cur["wv"][:, ec, msl], rhs=xt[:, ec, :tsz],
                            start=(ec == 0), stop=(ec == EC - 1),
                        )
                    store = sp.tile(
                        [P, 512], F32, tag="store", bufs=2, name=f"st{q}_{i}_{mt}"
                    )[:, :tsz]
                    nc.vector.tensor_mul(store, pvp, gates[mt])
                    init = (
                        0.0
                        if i == 0
                        else mem_prev[:, mt, BLK[i - 1][1] - 1 : BLK[i - 1][1]]
                    )
                    nc.vector.tensor_tensor_scan(
                        mem_t[:, mt, :tsz], decay_t[:, :tsz], store,
                        initial=init,
                        op0=mybir.AluOpType.mult, op1=mybir.AluOpType.add,
                    )

                prev = (
                    None
                    if i == 0
                    else dict(
                        q=q, i=i, tsz=tsz, mem=mem_t, xt=xt, x8=x8t,
                        q8=cur["q8"], q16=cur["q16"],
                        osl=slice(t0 - HALO, t0 - HALO + tsz),
                    )
                )
                mem_prev = mem_t

            emit_pq(prev)  # drain: pq for (3, B4) covers the C transition
            wo_t[1] = load_wo(1, "wv")

            # ---- Phase C: output projection, all Wo quarters resident ----
            lt = sp.tile([P, MT, 512], BF16, tag="xt", bufs=3, name="lt0")
            nc.sync.dma_start(out=lt, in_=l0_r[:, :, 0:512])
            for tb in range(OUT_T // 512):
                tsl = slice(tb * 512, (tb + 1) * 512)
                lt_next = None
                if tb + 1 < OUT_T // 512:
                    lt_next = sp.tile(
                        [P, MT, 512], BF16, tag="xt", bufs=3, name=f"lt{tb+1}"
                    )
                    nc.sync.dma_start(
                        out=lt_next, in_=l0_r[:, :, (tb + 1) * 512 : (tb + 2) * 512]
                    )
                for eq in range(4):
                    ot = sp.tile(
                        [P, MT_Q, 512], F32, tag="mem", bufs=2, name=f"ot{eq}_{tb}"
                    )
                    for et in range(MT_Q):
                        pop = ps.tile(
                            [P, 512], F32, tag="pv", bufs=3, name=f"po{eq}_{tb}_{et}"
                        )
                        for mc in range(MT):
                            nc.tensor.matmul(
                                pop,
                                lhsT=wo_t[eq][:, mc, et * P : (et + 1) * P],
                                rhs=lt[:, mc, :],
                                start=(mc == 0), stop=(mc == MT - 1),
                            )
                        nc.vector.tensor_copy(ot[:, et, :], pop)
                    nc.gpsimd.dma_start(
                        out=outT_r[:, eq * MT_Q : (eq + 1) * MT_Q, tsl], in_=ot
                    )
                lt = lt_next
    nc.compile()
    return nc


_cached = {}


def _get_module(has_bias):
    if has_bias not in _cached:
        _cached[has_bias] = build_module(has_bias)
    return _cached[has_bias]


def _q8(a):
    return np.clip(a * np.float32(XS), -240, 240).astype(ml_dtypes.float8_e4m3)


def _prep_inputs(x, Wv, Wg, bg, Wq, bq, Wo, has_bias):
    """Shard + quantize host-side. Returns per-core input dicts."""
    bf = ml_dtypes.bfloat16
    x = np.asarray(x, dtype=np.float32)
    Wv16 = (np.asarray(Wv, np.float32) * np.float32(SCALE)).astype(bf)
    Wo16 = (np.asarray(Wo, np.float32) * np.float32(SCALE)).astype(bf)
    Wg = np.asarray(Wg, np.float32)
    Wq = np.asarray(Wq, np.float32)
    Wg8, Wq8 = _q8(Wg[:KF]), _q8(Wq[:KF])
    Wg16 = (Wg[KF:] * np.float32(WS)).astype(bf)
    Wq16 = (Wq[KF:] * np.float32(WS)).astype(bf)
    in_maps = []
    for c in range(N_CORES):
        b, h = c // 2, c % 2
        xTc = np.zeros((E, T), dtype=np.float32)
        start = h * OUT_T - HALO
        src = np.ascontiguousarray(x[b, max(start, 0) : h * OUT_T + OUT_T].T)
        xTc[:, T - src.shape[1] :] = src
        m = {
            "xT16": xTc.astype(bf), "xT8": _q8(xTc[:KF]),
            "Wv16": Wv16, "Wg8": Wg8, "Wg16": Wg16,
            "Wq8": Wq8, "Wq16": Wq16, "Wo16": Wo16,
        }
        if has_bias:
            m["bg"] = np.ascontiguousarray(bg, dtype=np.float32)
            m["bq"] = np.ascontiguousarray(bq, dtype=np.float32)
        in_maps.append(m)
    return in_maps


def run(x, Wv, Wg, bg, Wq, bq, Wo, trace=False):
    bg = np.asarray(bg, dtype=np.float32)
    bq = np.asarray(bq, dtype=np.float32)
    has_bias = bool(np.any(bg)) or bool(np.any(bq))
    nc = _get_module(has_bias)
    in_maps = _prep_inputs(x, Wv, Wg, bg, Wq, bq, Wo, has_bias)
    res = run_bass_kernel_spmd(
        nc, in_maps, core_ids=list(range(N_CORES)), trace=trace
    )
    out = np.empty((B, S, E), dtype=np.float32)
    for c in range(N_CORES):
        b, h = c // 2, c % 2
        out[b, h * OUT_T : (h + 1) * OUT_T] = res.results[c]["outT"].T
    return out, res


def kernel(**inputs):
    out, _ = run(**inputs)
    return out
